# revision 18
# baseline (speedup 1.0000x reference)
"""Trainium2 Bass kernel for nn_Attention_2851858284976.

Dense transformer attention block, b=8 n=1024 dim=1024 heads=16.
Sharding: pure data parallel — one batch element per NeuronCore (8 cores).

Per-core math (batch element x of shape (n, dim)):
  Y = x @ w_qkv^T                              (n, 3*dim)
  Z = Y.reshape(49152, 64)   # raw reshape: rows are (token, col-block) pairs
  Q = Z[0:16384], K = Z[16384:32768], V = Z[32768:49152], each (16, 1024, 64)
  per head: P^T = exp(scale * K_h @ Q_h^T)     (softmax along the partition axis)
            [O^T; Zs*64] = [V_h | 1*64]^T @ P^T  (ones cols replicate the denom)
            oT_h = O^T * (1/Zs)
  out = (oT stacked).T @ w_out^T + b_out

Matmul datapath is bf16 (1 cyc/row on the PE); accumulation fp32 in PSUM.
Host pre-transposes x / w_qkv / w_out so the contraction dim lands on the
SBUF partition axis. Z is staged in DRAM padded to 128-wide rows so the
Q/K head blocks can be transposed by the DMA xbar engine (fast path needs
>=128 source columns) instead of burning TensorE transposes.
"""
import numpy as np
import ml_dtypes

import concourse.bass as bass
import concourse.mybir as mybir
from concourse import bacc
from concourse.tile import TileContext
from concourse.bass_utils import run_bass_kernel_spmd

N_CORES = 8
N = 1024          # tokens
DIM = 1024
E3 = 3 * DIM      # qkv projection width
H = 16            # heads
HD = 64           # head dim
SCALE = HD ** -0.5
ZROWS = N * E3 // HD          # 49152 Z-rows, padded to 128 wide in DRAM

F32 = mybir.dt.float32
BF = mybir.dt.bfloat16
FT = mybir.ActivationFunctionType


def build():
    nc = bacc.Bacc("TRN2", target_bir_lowering=False, num_devices=N_CORES)
    xt = nc.declare_dram_parameter("xt", [DIM, N], BF, isOutput=False)
    wqkvt = nc.declare_dram_parameter("wqkvt", [DIM, E3], BF, isOutput=False)
    woutt = nc.declare_dram_parameter("woutt", [DIM, DIM], BF, isOutput=False)
    bias = nc.declare_dram_parameter("bias", [1, DIM], F32, isOutput=False)
    outp = nc.declare_dram_parameter("out", [N, DIM], F32, isOutput=True)

    with TileContext(nc) as tc:
        with tc.tile_pool(name="dram", bufs=1, space="DRAM") as dpool, \
             tc.tile_pool(name="singles", bufs=1) as singles:
            zbuf = dpool.tile([ZROWS, 128], BF)    # Z rows, cols 64:128 unused
            zb3 = zbuf.rearrange("(r c) d -> r c d", c=48)   # (1024, 48, 128)

            oT = singles.tile([128, 8, N], BF)    # [64*(h%2)+dd, h//2, i]
            biasrep = singles.tile([128, DIM], F32)
            nc.sync.dma_start(out=biasrep, in_=bias[:].to_broadcast((128, DIM)))

            # [V | ones*64] stationary tiles for the PV matmul; ones half gives
            # the softmax denominator replicated on out rows 64-127.
            ones_f = singles.tile([128, 8, HD], F32)
            nc.vector.memset(ones_f, 1.0)
            vh0 = singles.tile([128, 8, 2 * HD], BF)
            vh1 = singles.tile([128, 8, 2 * HD], BF)
            vh2 = singles.tile([128, 8, 2 * HD], BF)
            vh3 = singles.tile([128, 8, 2 * HD], BF)
            vhs = [vh0, vh1, vh2, vh3]
            for v in vhs:
                nc.vector.tensor_copy(v[:, :, HD:2 * HD], ones_f)
                nc.vector.tensor_copy(v[:, :, 0:HD], ones_f)

            # Q/K transposed via DMA xbar: (4096,128) chunks of zbuf ->
            # QKall[:, c*4096:...]; rows 0-63 hold Zt data, 64-127 padding.
            with tc.tile_pool(name="qk", bufs=1) as qkpool:
                QKall = qkpool.tile([128, 32768], BF)

                # ---------- phase 1: Y = x @ w_qkv^T -> zbuf ----------
                # Transpose chunk c of the Q/K region only needs Y rows up to
                # (c+1)*4096/48; emit each right after the i-tile completing it
                # so the xbar runs during the phase-1 tail. zbuf writes go on
                # the gpsimd (SWDGE) queue so they don't queue ahead of the
                # transposes on sync.
                tr_after = {0: [0], 1: [1, 2], 2: [3], 3: [4, 5], 4: [6], 5: [7]}
                with tc.tile_pool(name="p1", bufs=1) as p1, \
                     tc.tile_pool(name="p1st", bufs=6) as p1st, \
                     tc.tile_pool(name="ps1", bufs=8, space="PSUM") as ps1:
                    XT = p1.tile([128, 8, N], BF)
                    WT = p1.tile([128, 8, E3], BF)
                    for kt in range(8):
                        nc.sync.dma_start(
                            out=XT[:, kt, :], in_=xt[kt * 128:(kt + 1) * 128, :])
                        nc.sync.dma_start(
                            out=WT[:, kt, :], in_=wqkvt[kt * 128:(kt + 1) * 128, :])
                    for it in range(8):
                        for ec in range(6):
                            ps = ps1.tile([128, 512], F32)
                            nc.tensor.matmul(
                                ps[0:128, 0:128],
                                lhsT=vh0[:, 0, :], rhs=vh0[:, 0, :],
                                start=True, stop=True)
                            for kt in range(8):
                                nc.tensor.matmul(
                                    ps,
                                    lhsT=XT[:, kt, it * 128:(it + 1) * 128],
                                    rhs=WT[:, kt, ec * 512:(ec + 1) * 512],
                                    start=(kt == 0), stop=(kt == 7))
                            st = p1st.tile([128, 512], BF)
                            nc.scalar.copy(st, ps)
                            nc.sync.dma_start(
                                out=zb3[it * 128:(it + 1) * 128,
                                        ec * 8:(ec + 1) * 8, 0:HD],
                                in_=st.rearrange("p (b d) -> p b d", d=HD))

                for c in range(16):
                    nc.sync.dma_start_transpose(
                        QKall[:, c * 2048:(c + 1) * 2048],
                        zbuf[c * 2048:(c + 1) * 2048, :])

                def qt_sl(h, lo, sz):
                    return QKall[0:64, h * N + lo: h * N + lo + sz]

                def kt_sl(h, lo, sz):
                    return QKall[0:64, 16384 + h * N + lo: 16384 + h * N + lo + sz]

                with tc.tile_pool(name="p3", bufs=1) as p3:
                    WOT = p3.tile([128, 8, DIM], BF)
                    nc.sync.dma_start(
                        out=WOT, in_=woutt[:].rearrange("(a p) e -> p a e", p=128))

                    # ---------- phase 2: attention per head ----------
                    with tc.tile_pool(name="pt", bufs=6) as ptpool, \
                         tc.tile_pool(name="rz", bufs=4) as rzpool, \
                         tc.tile_pool(name="sps", bufs=3, space="PSUM") as spsum, \
                         tc.tile_pool(name="ops", bufs=1, space="PSUM") as opsum:
                        for h in range(H):
                            po, hf = 64 * (h % 2), h // 2
                            vh = vhs[h % 4]
                            nc.sync.dma_start(
                                out=vh[:, :, 0:HD],
                                in_=zbuf[32768 + h * N: 32768 + (h + 1) * N, 0:HD
                                         ].rearrange("(t p) d -> p t d", p=128))
                            ops0 = opsum.tile([128, 512], F32, tag="ops0")
                            ops1 = opsum.tile([128, 512], F32, tag="ops1")
                            ops = (ops0, ops1)
                            warm = vhs[(h + 2) % 4]
                            for jt in range(8):
                                sps = spsum.tile([128, 2, 512], F32, tag="sps")
                                # keep the PE HAM window busy across the
                                # ACT-bound stall; output is overwritten by the
                                # real scores matmul (start=True).
                                nc.tensor.matmul(
                                    sps[0:128, 0, 0:128],
                                    lhsT=warm[:, 0, :], rhs=warm[:, 0, :],
                                    start=True, stop=True)
                                for ic in range(2):
                                    nc.tensor.matmul(
                                        sps[:, ic, :],
                                        lhsT=kt_sl(h, jt * 128, 128),
                                        rhs=qt_sl(h, ic * 512, 512),
                                        start=True, stop=True)
                                pt = ptpool.tile([128, 2, 512], BF, tag="pt")
                                nc.scalar.activation(pt, sps, FT.Exp, scale=SCALE)
                                for ic in range(2):
                                    nc.tensor.matmul(
                                        ops[ic],
                                        lhsT=vh[:, jt, :],
                                        rhs=pt[:, ic, :],
                                        start=(jt == 0), stop=(jt == 7),
                                        skip_group_check=True)
                            for ic in range(2):
                                # custom-DVE reciprocal can't read PSUM; stage
                                # the denominator rows through SBUF first.
                                zst = rzpool.tile([64, 512], F32, tag="zst")
                                nc.vector.tensor_copy(zst, ops[ic][64:128, :])
                                rzs = rzpool.tile([64, 512], F32, tag="rzs")
                                nc.vector.reciprocal_approx_fast(rzs, zst)
                                nc.vector.tensor_mul(
                                    oT[po:po + 64, hf, ic * 512:(ic + 1) * 512],
                                    ops[ic][0:64, :], rzs)

                    # ---------- phase 3: out = oT.T @ w_out^T + b ----------
                    with tc.tile_pool(name="p3st", bufs=4) as p3st, \
                         tc.tile_pool(name="ps3", bufs=4, space="PSUM") as ps3:
                        for it in range(8):
                            for ec in range(2):
                                rps = ps3.tile([128, 512], F32)
                                for ct in range(8):
                                    nc.tensor.matmul(
                                        rps,
                                        lhsT=oT[:, ct, it * 128:(it + 1) * 128],
                                        rhs=WOT[:, ct, ec * 512:(ec + 1) * 512],
                                        start=(ct == 0), stop=(ct == 7))
                                ost = p3st.tile([128, 512], F32)
                                nc.vector.tensor_add(
                                    ost, rps, biasrep[:, ec * 512:(ec + 1) * 512])
                                nc.sync.dma_start(
                                    out=outp[it * 128:(it + 1) * 128,
                                             ec * 512:(ec + 1) * 512],
                                    in_=ost)

    nc.finalize()
    return nc


_CACHE = {}


def _get_nc():
    if "nc" not in _CACHE:
        _CACHE["nc"] = build()
    return _CACHE["nc"]


def make_in_maps(x, w_qkv, w_out, b_out):
    bf = ml_dtypes.bfloat16
    wqkvt = np.ascontiguousarray(np.asarray(w_qkv, dtype=np.float32).T).astype(bf)
    woutt = np.ascontiguousarray(np.asarray(w_out, dtype=np.float32).T).astype(bf)
    bias = np.ascontiguousarray(np.asarray(b_out, dtype=np.float32).reshape(1, DIM))
    x = np.asarray(x, dtype=np.float32)
    return [
        {
            "xt": np.ascontiguousarray(x[b].T).astype(bf),
            "wqkvt": wqkvt,
            "woutt": woutt,
            "bias": bias,
        }
        for b in range(N_CORES)
    ]


def kernel(x, w_qkv, w_out, b_out):
    nc = _get_nc()
    in_maps = make_in_maps(x, w_qkv, w_out, b_out)
    res = run_bass_kernel_spmd(nc, in_maps, core_ids=list(range(N_CORES)))
    return np.stack(
        [res.results[b]["out"] for b in range(N_CORES)], axis=0
    ).astype(np.float32)
